# revision 16
# baseline (speedup 1.0000x reference)
"""Trainium2 Bass kernel for DirectionalFreqEmbed (per-token gather + grouped GEMM).

Token-parallel across 8 NeuronCores, one compiled program per core, tokens
greedy-balanced by chunk count. The host shards the inputs into per-core
operand panels: for each token the gathered x values are packed densely into
ceil((len+1)/128) chunks of 128 l-slots ([128, 64] bf16 panels, batch on the
free axis, plus a ones-slot that folds the bias into the GEMM), and the
per-token W rows are permuted to match ([128, 384] bf16 per chunk, zero rows
on padding). The device program is then a pure streaming block-GEMM: per
token one W-tile DMA and C_t accumulated bf16 matmuls into PSUM, a bf16
cast-copy, and a store. W is read exactly once at its true ragged size
(sum(lens) rows, ~97 MB chip-wide instead of the 283 MB dense padding).

kernel(**inputs) takes FULL unsharded inputs and returns the FULL output.
"""
import os
import sys

import ml_dtypes
import numpy as np

for _p in ("/opt/trn_rl_repo", "/root/.axon_site/_ro/trn_rl_repo"):
    if os.path.isdir(_p) and _p not in sys.path:
        sys.path.insert(0, _p)

try:  # the staged antenv lacks axon_hooks; inject a functional stand-in
    import antenv.axon_hooks  # noqa: F401
except ImportError:
    import types as _types

    _hooks = _types.ModuleType("antenv.axon_hooks")
    _hooks._hook = None
    _hooks.get_axon_ntff_profile_hook = lambda: _hooks._hook
    _hooks.set_axon_ntff_profile_hook = lambda h: setattr(_hooks, "_hook", h)
    sys.modules["antenv.axon_hooks"] = _hooks

import jax
import concourse.bass as bass  # noqa: F401
import concourse.tile as tile
from concourse import bacc, mybir

IMG, CIN, DIM, B = 64, 30, 384, 64
T, Lmax = 240, 1452
NCORES = 8

bf16 = mybir.dt.bfloat16
f32 = mybir.dt.float32

_cache = {}


def _assign_tokens(lens):
    """Greedy LPT balance of tokens across cores by chunk count."""
    C = np.ceil((lens.astype(np.int64) + 1) / 128).astype(np.int64)
    order = np.argsort(-C, kind="stable")
    loads = [0] * NCORES
    toks = [[] for _ in range(NCORES)]
    for t in order:
        k = min(range(NCORES), key=lambda k: (loads[k], len(toks[k])))
        toks[k].append(int(t))
        loads[k] += int(C[t])
    # per-core order: 3 smallest first (fast pipeline start), then the rest
    # descending so the final token is small (short post-DMA tail chain).
    out = []
    for tk in toks:
        s = sorted(tk, key=lambda t: int(C[t]))
        out.append(s[:3] + s[3:][::-1])
    return out, C


def _shard(x, W, bias, idx_a, idx_b, idx_c, lens):
    """Per-core slot stream: each token padded to whole 128-slot chunks
    (PE base-partition rule requires chunk starts at partition 0), packed
    into [128, *] panels for full SBUF residency."""
    tok_lists, C = _assign_tokens(lens)
    in_maps, plans = [], []
    xbf = x.astype(np.float32)
    for k in range(NCORES):
        toks = tok_lists[k]
        nchunks = int(sum(C[t] for t in toks))
        xg = np.zeros((nchunks * 128, B), np.float32)
        wg = np.zeros((nchunks * 128, DIM), np.float32)
        plan = []
        s0 = 0
        for t in toks:
            L = int(lens[t])
            c = int(C[t])
            g = xbf[:, idx_c[t, :L], idx_a[t, :L], idx_b[t, :L]]  # [B, L]
            xg[s0:s0 + L] = g.T
            xg[s0 + L] = 1.0  # ones-slot -> bias row
            wg[s0:s0 + L] = W[t, :L]
            wg[s0 + L] = bias[t]
            plan.append(c)
            s0 += c * 128
        x_core = np.ascontiguousarray(
            xg.reshape(nchunks, 128, B).transpose(1, 0, 2).reshape(128, -1)
        ).astype(ml_dtypes.bfloat16)
        w_core = np.ascontiguousarray(
            wg.reshape(nchunks, 128, DIM).transpose(1, 0, 2).reshape(128, -1)
        ).astype(ml_dtypes.bfloat16)
        in_maps.append({"x_core": x_core, "w_core": w_core})
        plans.append(plan)
    return in_maps, plans, tok_lists


def _build_program(plan):
    from contextlib import ExitStack

    tpc = len(plan)
    nchunks = sum(plan)

    # W/x panels are fully SBUF-resident, loaded in a few large growing
    # pieces (piece boundaries on token boundaries) so PE can start early
    # while DMA streams at near line rate.
    tok_chunk0 = []
    c0 = 0
    for c in plan:
        tok_chunk0.append(c0)
        c0 += c
    # growing piece sizes for an early start, shrinking at the end so the
    # final matmul burst trails the stream tightly
    tail_chunks = min(16, nchunks)
    sizes = []
    want, left = 2, nchunks - tail_chunks
    while left > 0:
        s = min(want, left)
        sizes.append(s)
        left -= s
        want = min(want * 2, 24)
    sizes += [8] * (tail_chunks // 8) + ([tail_chunks % 8] if tail_chunks % 8 else [])
    # snap piece boundaries to token boundaries (chunks of one token stay in
    # one piece): walk tokens, cut when the current piece reaches its target
    bounds = np.cumsum(sizes)
    pieces, cur0, acc, bi = [], 0, 0, 0
    for j, c in enumerate(plan):
        acc += c
        if (bi < len(bounds) and cur0 + acc >= bounds[bi]) or j == tpc - 1:
            pieces.append((cur0, acc))
            cur0 += acc
            acc = 0
            while bi < len(bounds) and bounds[bi] <= cur0:
                bi += 1
    piece_of_chunk = {}
    for pi, (p0, n) in enumerate(pieces):
        for cc in range(p0, p0 + n):
            piece_of_chunk[cc] = pi

    nc = bacc.Bacc("TRN2", target_bir_lowering=False, debug=False, num_devices=1)
    x_core = nc.dram_tensor("x_core", [128, nchunks * B], bf16,
                            kind="ExternalInput").ap()
    w_core = nc.dram_tensor("w_core", [128, nchunks * DIM], bf16,
                            kind="ExternalInput").ap()
    y_core = nc.dram_tensor("y_core", [tpc, B, DIM], bf16, kind="ExternalOutput").ap()

    with tile.TileContext(nc) as tc, ExitStack() as ctx:
        sb_pool = ctx.enter_context(tc.tile_pool(name="sb", bufs=1))
        ps_pool = ctx.enter_context(tc.tile_pool(name="ps", bufs=8, space="PSUM"))
        out_pool = ctx.enter_context(tc.tile_pool(name="o", bufs=4))

        # alternate W pieces across the two HWDGE rings (a single ring tops
        # out near ~290 GB/s); each piece's x rides the opposite ring.
        x_tiles, w_tiles = [], []
        for pi, (p0, n) in enumerate(pieces):
            wring = nc.sync if pi % 2 == 0 else nc.scalar
            xring = nc.scalar if pi % 2 == 0 else nc.sync
            wt = sb_pool.tile([128, n * DIM], bf16, tag=f"w{pi}")
            wring.dma_start(wt[:], w_core[:, p0 * DIM:(p0 + n) * DIM])
            xt = sb_pool.tile([128, n * B], bf16, tag=f"x{pi}")
            xring.dma_start(xt[:], x_core[:, p0 * B:(p0 + n) * B])
            x_tiles.append(xt)
            w_tiles.append(wt)

        for j, c in enumerate(plan):
            psum = ps_pool.tile([B, DIM], f32)
            for ck in range(c):
                cc = tok_chunk0[j] + ck
                pi = piece_of_chunk[cc]
                lc = cc - pieces[pi][0]
                nc.tensor.matmul(
                    psum[:],
                    lhsT=x_tiles[pi][:, lc * B:(lc + 1) * B],
                    rhs=w_tiles[pi][:, lc * DIM:(lc + 1) * DIM],
                    start=(ck == 0),
                    stop=(ck == c - 1),
                )
            o_tile = out_pool.tile([B, DIM], bf16)
            nc.vector.tensor_copy(o_tile[:], psum[:])
            oring = nc.scalar if j % 2 == 0 else nc.sync
            oring.dma_start(y_core[j], o_tile[:])

    nc.compile()
    return nc


def _run_per_core(ncs, in_maps):
    """Per-device execution of 8 distinct single-core programs (adapted from
    bass2jax.run_bass_via_pjrt's single-core path)."""
    from concurrent.futures import ThreadPoolExecutor

    from concourse import mybir as mb
    from concourse.bass2jax import _bass_exec_p, install_neuronx_cc_hook

    install_neuronx_cc_hook()
    devices = jax.devices()[:8]

    def launch(k):
        nc = ncs[k]
        in_names, out_names, out_avals, zero_outs = [], [], [], []
        for alloc in nc.m.functions[0].allocations:
            if not isinstance(alloc, mb.MemoryLocationSet):
                continue
            name = alloc.memorylocations[0].name
            if alloc.kind == "ExternalInput":
                in_names.append(name)
            elif alloc.kind == "ExternalOutput":
                shape = tuple(alloc.tensor_shape)
                dtype = mb.dt.np(alloc.dtype)
                out_names.append(name)
                out_avals.append(jax.core.ShapedArray(shape, dtype))
                zero_outs.append(np.zeros(shape, dtype))
        n_params = len(in_names)
        all_names = tuple(in_names + out_names)
        donate = tuple(range(n_params, n_params + len(out_names)))

        def _body(*args):
            outs = _bass_exec_p.bind(
                *args,
                out_avals=tuple(out_avals),
                in_names=all_names,
                out_names=tuple(out_names),
                lowering_input_output_aliases=(),
                sim_require_finite=True,
                sim_require_nnan=True,
                nc=nc,
            )
            return tuple(outs)

        dev = devices[k]
        extras = {}
        for alloc in nc.m.functions[0].allocations:
            if (isinstance(alloc, mb.MemoryLocationSet)
                    and alloc.kind == "ExternalInput"):
                name = alloc.memorylocations[0].name
                if name not in in_maps[k]:
                    extras[name] = np.full(
                        tuple(alloc.tensor_shape), k, mb.dt.np(alloc.dtype))
        args = [jax.device_put(np.asarray(in_maps[k].get(n, extras.get(n))), dev)
                for n in in_names]
        args += [jax.device_put(z, dev) for z in zero_outs]
        out_arrs = jax.jit(_body, donate_argnums=donate, keep_unused=True)(*args)
        return out_names, out_arrs

    with ThreadPoolExecutor(max_workers=8) as ex:
        futs = [ex.submit(launch, k) for k in range(8)]
        handles = [f.result() for f in futs]
    return [
        {name: np.asarray(arr) for name, arr in zip(names, arrs)}
        for names, arrs in handles
    ]


LAST_RESULTS = None


def kernel(x, W, bias, idx_a, idx_b, idx_c, lens):
    global LAST_RESULTS
    x = np.asarray(x, np.float32)
    W = np.asarray(W, np.float32)
    bias = np.asarray(bias, np.float32)
    idx_a = np.asarray(idx_a, np.int32)
    idx_b = np.asarray(idx_b, np.int32)
    idx_c = np.asarray(idx_c, np.int32)
    lens = np.asarray(lens, np.int32)
    assert x.shape == (B, CIN, IMG, IMG) and W.shape == (T, Lmax, DIM)

    in_maps, plans, tok_lists = _shard(x, W, bias, idx_a, idx_b, idx_c, lens)
    if "ncs" not in _cache:
        _cache["ncs"] = [_build_program(plans[k]) for k in range(NCORES)]
    ncs = _cache["ncs"]

    hook = None
    trace = os.environ.get("BASS_TRACE") and not os.environ.get("BASS_NEVER_TRACE")
    if trace:
        from antenv.axon_hooks import get_axon_ntff_profile_hook

        hook = get_axon_ntff_profile_hook()
    if hook is not None:
        tmpdir = os.environ.get("KERNEL_TRACE_TMPDIR") or "/tmp/kernel_trace"
        os.makedirs(tmpdir, exist_ok=True)
        with hook(tmpdir, [0]):
            results = _run_per_core(ncs, in_maps)
        LAST_RESULTS = ("ntff", tmpdir, ncs[0])
    else:
        results = _run_per_core(ncs, in_maps)
        LAST_RESULTS = None

    y = np.empty((B, T, DIM), np.float32)
    for k in range(NCORES):
        y[:, tok_lists[k], :] = results[k]["y_core"].transpose(1, 0, 2).astype(
            np.float32)
    return y


# revision 18
# speedup vs baseline: 1.0102x; 1.0102x over previous
"""Trainium2 Bass kernel for DirectionalFreqEmbed (per-token gather + grouped GEMM).

Token-parallel across 8 NeuronCores, one compiled program per core, tokens
greedy-balanced by chunk count. The host shards the inputs into per-core
operand panels: for each token the gathered x values are packed densely into
ceil((len+1)/128) chunks of 128 l-slots ([128, 64] bf16 panels, batch on the
free axis, plus a ones-slot that folds the bias into the GEMM), and the
per-token W rows are permuted to match ([128, 384] bf16 per chunk, zero rows
on padding). The device program is then a pure streaming block-GEMM: per
token one W-tile DMA and C_t accumulated bf16 matmuls into PSUM, a bf16
cast-copy, and a store. W is read exactly once at its true ragged size
(sum(lens) rows, ~97 MB chip-wide instead of the 283 MB dense padding).

kernel(**inputs) takes FULL unsharded inputs and returns the FULL output.
"""
import os
import sys

import ml_dtypes
import numpy as np

for _p in ("/opt/trn_rl_repo", "/root/.axon_site/_ro/trn_rl_repo"):
    if os.path.isdir(_p) and _p not in sys.path:
        sys.path.insert(0, _p)

try:  # the staged antenv lacks axon_hooks; inject a functional stand-in
    import antenv.axon_hooks  # noqa: F401
except ImportError:
    import types as _types

    _hooks = _types.ModuleType("antenv.axon_hooks")
    _hooks._hook = None
    _hooks.get_axon_ntff_profile_hook = lambda: _hooks._hook
    _hooks.set_axon_ntff_profile_hook = lambda h: setattr(_hooks, "_hook", h)
    sys.modules["antenv.axon_hooks"] = _hooks

import jax
import concourse.bass as bass  # noqa: F401
import concourse.tile as tile
from concourse import bacc, mybir

IMG, CIN, DIM, B = 64, 30, 384, 64
T, Lmax = 240, 1452
NCORES = 8

bf16 = mybir.dt.bfloat16
f32 = mybir.dt.float32

_cache = {}


def _assign_tokens(lens):
    """Greedy LPT balance of tokens across cores by chunk count."""
    C = np.ceil((lens.astype(np.int64) + 1) / 128).astype(np.int64)
    order = np.argsort(-C, kind="stable")
    loads = [0] * NCORES
    toks = [[] for _ in range(NCORES)]
    for t in order:
        k = min(range(NCORES), key=lambda k: (loads[k], len(toks[k])))
        toks[k].append(int(t))
        loads[k] += int(C[t])
    # per-core order: 3 smallest first (fast pipeline start), then the rest
    # descending so the final token is small (short post-DMA tail chain).
    out = []
    for tk in toks:
        s = sorted(tk, key=lambda t: int(C[t]))
        out.append(s[:3] + s[3:][::-1])
    return out, C


def _shard(x, W, bias, idx_a, idx_b, idx_c, lens):
    """Per-core slot stream: each token padded to whole 128-slot chunks
    (PE base-partition rule requires chunk starts at partition 0), packed
    into [128, *] panels for full SBUF residency."""
    tok_lists, C = _assign_tokens(lens)
    in_maps, plans = [], []
    xbf = x.astype(np.float32)
    for k in range(NCORES):
        toks = tok_lists[k]
        nchunks = int(sum(C[t] for t in toks))
        xg = np.zeros((nchunks * 128, B), np.float32)
        wg = np.zeros((nchunks * 128, DIM), np.float32)
        plan = []
        s0 = 0
        for t in toks:
            L = int(lens[t])
            c = int(C[t])
            g = xbf[:, idx_c[t, :L], idx_a[t, :L], idx_b[t, :L]]  # [B, L]
            xg[s0:s0 + L] = g.T
            xg[s0 + L] = 1.0  # ones-slot -> bias row
            wg[s0:s0 + L] = W[t, :L]
            wg[s0 + L] = bias[t]
            plan.append(c)
            s0 += c * 128
        x_core = np.ascontiguousarray(
            xg.reshape(nchunks, 128, B).transpose(1, 0, 2).reshape(128, -1)
        ).astype(ml_dtypes.bfloat16)
        w_core = np.ascontiguousarray(
            wg.reshape(nchunks, 128, DIM).transpose(1, 0, 2).reshape(128, -1)
        ).astype(ml_dtypes.bfloat16)
        in_maps.append({"x_core": x_core, "w_core": w_core})
        plans.append(plan)
    return in_maps, plans, tok_lists


def _build_program(plan):
    from contextlib import ExitStack

    tpc = len(plan)
    nchunks = sum(plan)

    # W/x panels are fully SBUF-resident, loaded in a few large growing
    # pieces (piece boundaries on token boundaries) so PE can start early
    # while DMA streams at near line rate.
    tok_chunk0 = []
    c0 = 0
    for c in plan:
        tok_chunk0.append(c0)
        c0 += c
    # uniform ~8-chunk pieces, snapped to token boundaries: with strict ring
    # alternation the two rings deliver pieces nearly in lockstep, so the
    # in-order token consumption never waits long for a late piece.
    pieces, cur0, acc = [], 0, 0
    for j, c in enumerate(plan):
        acc += c
        if acc >= 8 or j == tpc - 1:
            pieces.append((cur0, acc))
            cur0 += acc
            acc = 0
    piece_of_chunk = {}
    for pi, (p0, n) in enumerate(pieces):
        for cc in range(p0, p0 + n):
            piece_of_chunk[cc] = pi

    nc = bacc.Bacc("TRN2", target_bir_lowering=False, debug=False, num_devices=1)
    x_core = nc.dram_tensor("x_core", [128, nchunks * B], bf16,
                            kind="ExternalInput").ap()
    w_core = nc.dram_tensor("w_core", [128, nchunks * DIM], bf16,
                            kind="ExternalInput").ap()
    y_core = nc.dram_tensor("y_core", [tpc, B, DIM], bf16, kind="ExternalOutput").ap()

    with tile.TileContext(nc) as tc, ExitStack() as ctx:
        sb_pool = ctx.enter_context(tc.tile_pool(name="sb", bufs=1))
        ps_pool = ctx.enter_context(tc.tile_pool(name="ps", bufs=8, space="PSUM"))
        out_pool = ctx.enter_context(tc.tile_pool(name="o", bufs=4))

        # alternate W pieces across the two HWDGE rings (a single ring tops
        # out near ~290 GB/s); each piece's x rides the opposite ring.
        x_tiles, w_tiles = [], []
        for pi, (p0, n) in enumerate(pieces):
            wring = nc.sync if pi % 2 == 0 else nc.scalar
            xring = nc.scalar if pi % 2 == 0 else nc.sync
            wt = sb_pool.tile([128, n * DIM], bf16, tag=f"w{pi}")
            wring.dma_start(wt[:], w_core[:, p0 * DIM:(p0 + n) * DIM])
            xt = sb_pool.tile([128, n * B], bf16, tag=f"x{pi}")
            xring.dma_start(xt[:], x_core[:, p0 * B:(p0 + n) * B])
            x_tiles.append(xt)
            w_tiles.append(wt)

        for j, c in enumerate(plan):
            psum = ps_pool.tile([B, DIM], f32)
            for ck in range(c):
                cc = tok_chunk0[j] + ck
                pi = piece_of_chunk[cc]
                lc = cc - pieces[pi][0]
                nc.tensor.matmul(
                    psum[:],
                    lhsT=x_tiles[pi][:, lc * B:(lc + 1) * B],
                    rhs=w_tiles[pi][:, lc * DIM:(lc + 1) * DIM],
                    start=(ck == 0),
                    stop=(ck == c - 1),
                )
            o_tile = out_pool.tile([B, DIM], bf16)
            if j % 2 == 0:
                nc.vector.tensor_copy(o_tile[:], psum[:])
            else:
                nc.scalar.copy(o_tile[:], psum[:])
            oring = nc.scalar if j % 2 == 0 else nc.sync
            oring.dma_start(y_core[j], o_tile[:])

    nc.compile()
    return nc


def _run_per_core(ncs, in_maps):
    """Per-device execution of 8 distinct single-core programs (adapted from
    bass2jax.run_bass_via_pjrt's single-core path)."""
    from concurrent.futures import ThreadPoolExecutor

    from concourse import mybir as mb
    from concourse.bass2jax import _bass_exec_p, install_neuronx_cc_hook

    install_neuronx_cc_hook()
    devices = jax.devices()[:8]

    def launch(k):
        nc = ncs[k]
        in_names, out_names, out_avals, zero_outs = [], [], [], []
        for alloc in nc.m.functions[0].allocations:
            if not isinstance(alloc, mb.MemoryLocationSet):
                continue
            name = alloc.memorylocations[0].name
            if alloc.kind == "ExternalInput":
                in_names.append(name)
            elif alloc.kind == "ExternalOutput":
                shape = tuple(alloc.tensor_shape)
                dtype = mb.dt.np(alloc.dtype)
                out_names.append(name)
                out_avals.append(jax.core.ShapedArray(shape, dtype))
                zero_outs.append(np.zeros(shape, dtype))
        n_params = len(in_names)
        all_names = tuple(in_names + out_names)
        donate = tuple(range(n_params, n_params + len(out_names)))

        def _body(*args):
            outs = _bass_exec_p.bind(
                *args,
                out_avals=tuple(out_avals),
                in_names=all_names,
                out_names=tuple(out_names),
                lowering_input_output_aliases=(),
                sim_require_finite=True,
                sim_require_nnan=True,
                nc=nc,
            )
            return tuple(outs)

        dev = devices[k]
        extras = {}
        for alloc in nc.m.functions[0].allocations:
            if (isinstance(alloc, mb.MemoryLocationSet)
                    and alloc.kind == "ExternalInput"):
                name = alloc.memorylocations[0].name
                if name not in in_maps[k]:
                    extras[name] = np.full(
                        tuple(alloc.tensor_shape), k, mb.dt.np(alloc.dtype))
        args = [jax.device_put(np.asarray(in_maps[k].get(n, extras.get(n))), dev)
                for n in in_names]
        args += [jax.device_put(z, dev) for z in zero_outs]
        out_arrs = jax.jit(_body, donate_argnums=donate, keep_unused=True)(*args)
        return out_names, out_arrs

    with ThreadPoolExecutor(max_workers=8) as ex:
        futs = [ex.submit(launch, k) for k in range(8)]
        handles = [f.result() for f in futs]
    return [
        {name: np.asarray(arr) for name, arr in zip(names, arrs)}
        for names, arrs in handles
    ]


LAST_RESULTS = None


def kernel(x, W, bias, idx_a, idx_b, idx_c, lens):
    global LAST_RESULTS
    x = np.asarray(x, np.float32)
    W = np.asarray(W, np.float32)
    bias = np.asarray(bias, np.float32)
    idx_a = np.asarray(idx_a, np.int32)
    idx_b = np.asarray(idx_b, np.int32)
    idx_c = np.asarray(idx_c, np.int32)
    lens = np.asarray(lens, np.int32)
    assert x.shape == (B, CIN, IMG, IMG) and W.shape == (T, Lmax, DIM)

    in_maps, plans, tok_lists = _shard(x, W, bias, idx_a, idx_b, idx_c, lens)
    if "ncs" not in _cache:
        _cache["ncs"] = [_build_program(plans[k]) for k in range(NCORES)]
    ncs = _cache["ncs"]

    hook = None
    trace = os.environ.get("BASS_TRACE") and not os.environ.get("BASS_NEVER_TRACE")
    if trace:
        from antenv.axon_hooks import get_axon_ntff_profile_hook

        hook = get_axon_ntff_profile_hook()
    if hook is not None:
        tmpdir = os.environ.get("KERNEL_TRACE_TMPDIR") or "/tmp/kernel_trace"
        os.makedirs(tmpdir, exist_ok=True)
        with hook(tmpdir, [0]):
            results = _run_per_core(ncs, in_maps)
        LAST_RESULTS = ("ntff", tmpdir, ncs[0])
    else:
        results = _run_per_core(ncs, in_maps)
        LAST_RESULTS = None

    y = np.empty((B, T, DIM), np.float32)
    for k in range(NCORES):
        y[:, tok_lists[k], :] = results[k]["y_core"].transpose(1, 0, 2).astype(
            np.float32)
    return y


# revision 20
# speedup vs baseline: 1.0528x; 1.0422x over previous
"""Trainium2 Bass kernel for DirectionalFreqEmbed (per-token gather + grouped GEMM).

Token-parallel across 8 NeuronCores, one compiled program per core, tokens
greedy-balanced by chunk count. The host shards the inputs into per-core
operand panels: for each token the gathered x values are packed densely into
ceil((len+1)/128) chunks of 128 l-slots ([128, 64] bf16 panels, batch on the
free axis, plus a ones-slot that folds the bias into the GEMM), and the
per-token W rows are permuted to match ([128, 384] bf16 per chunk, zero rows
on padding). The device program is then a pure streaming block-GEMM: per
token one W-tile DMA and C_t accumulated bf16 matmuls into PSUM, a bf16
cast-copy, and a store. W is read exactly once at its true ragged size
(sum(lens) rows, ~97 MB chip-wide instead of the 283 MB dense padding).

kernel(**inputs) takes FULL unsharded inputs and returns the FULL output.
"""
import os
import sys

import ml_dtypes
import numpy as np

for _p in ("/opt/trn_rl_repo", "/root/.axon_site/_ro/trn_rl_repo"):
    if os.path.isdir(_p) and _p not in sys.path:
        sys.path.insert(0, _p)

try:  # the staged antenv lacks axon_hooks; inject a functional stand-in
    import antenv.axon_hooks  # noqa: F401
except ImportError:
    import types as _types

    _hooks = _types.ModuleType("antenv.axon_hooks")
    _hooks._hook = None
    _hooks.get_axon_ntff_profile_hook = lambda: _hooks._hook
    _hooks.set_axon_ntff_profile_hook = lambda h: setattr(_hooks, "_hook", h)
    sys.modules["antenv.axon_hooks"] = _hooks

import jax
import concourse.bass as bass  # noqa: F401
import concourse.tile as tile
from concourse import bacc, mybir

IMG, CIN, DIM, B = 64, 30, 384, 64
T, Lmax = 240, 1452
NCORES = 8

bf16 = mybir.dt.bfloat16
f32 = mybir.dt.float32

_cache = {}


def _assign_tokens(lens):
    """Greedy LPT balance of tokens across cores by chunk count."""
    C = np.ceil((lens.astype(np.int64) + 1) / 128).astype(np.int64)
    order = np.argsort(-C, kind="stable")
    loads = [0] * NCORES
    toks = [[] for _ in range(NCORES)]
    for t in order:
        k = min(range(NCORES), key=lambda k: (loads[k], len(toks[k])))
        toks[k].append(int(t))
        loads[k] += int(C[t])
    # per-core order: 3 smallest first (fast pipeline start), then the rest
    # descending so the final token is small (short post-DMA tail chain).
    out = []
    for tk in toks:
        s = sorted(tk, key=lambda t: int(C[t]))
        out.append(s[:3] + s[3:][::-1])
    return out, C


def _shard(x, W, bias, idx_a, idx_b, idx_c, lens):
    """Per-core slot stream: each token padded to whole 128-slot chunks
    (PE base-partition rule requires chunk starts at partition 0), packed
    into [128, *] panels for full SBUF residency."""
    tok_lists, C = _assign_tokens(lens)
    in_maps, plans = [], []
    xbf = x.astype(np.float32)
    for k in range(NCORES):
        toks = tok_lists[k]
        nchunks = int(sum(C[t] for t in toks))
        xg = np.zeros((nchunks * 128, B), np.float32)
        wg = np.zeros((nchunks * 128, DIM), np.float32)
        plan = []
        s0 = 0
        for t in toks:
            L = int(lens[t])
            c = int(C[t])
            g = xbf[:, idx_c[t, :L], idx_a[t, :L], idx_b[t, :L]]  # [B, L]
            xg[s0:s0 + L] = g.T
            xg[s0 + L] = 1.0  # ones-slot -> bias row
            wg[s0:s0 + L] = W[t, :L]
            wg[s0 + L] = bias[t]
            plan.append(c)
            s0 += c * 128
        x_core = np.ascontiguousarray(
            xg.reshape(nchunks, 128, B).transpose(1, 0, 2).reshape(128, -1)
        ).astype(ml_dtypes.bfloat16)
        w_core = np.ascontiguousarray(
            wg.reshape(nchunks, 128, DIM).transpose(1, 0, 2).reshape(128, -1)
        ).astype(ml_dtypes.bfloat16)
        in_maps.append({"x_core": x_core, "w_core": w_core})
        plans.append(plan)
    return in_maps, plans, tok_lists


def _build_program(plan):
    from contextlib import ExitStack

    tpc = len(plan)
    nchunks = sum(plan)

    # W/x panels are fully SBUF-resident, loaded in a few large growing
    # pieces (piece boundaries on token boundaries) so PE can start early
    # while DMA streams at near line rate.
    tok_chunk0 = []
    c0 = 0
    for c in plan:
        tok_chunk0.append(c0)
        c0 += c
    # uniform ~8-chunk pieces, snapped to token boundaries: with strict ring
    # alternation the two rings deliver pieces nearly in lockstep, so the
    # in-order token consumption never waits long for a late piece.
    pieces, cur0, acc = [], 0, 0
    for j, c in enumerate(plan):
        acc += c
        want = 3 if len(pieces) < 2 else 8
        if acc >= want or j == tpc - 1:
            pieces.append((cur0, acc))
            cur0 += acc
            acc = 0
    piece_of_chunk = {}
    for pi, (p0, n) in enumerate(pieces):
        for cc in range(p0, p0 + n):
            piece_of_chunk[cc] = pi

    nc = bacc.Bacc("TRN2", target_bir_lowering=False, debug=False, num_devices=1)
    x_core = nc.dram_tensor("x_core", [128, nchunks * B], bf16,
                            kind="ExternalInput").ap()
    w_core = nc.dram_tensor("w_core", [128, nchunks * DIM], bf16,
                            kind="ExternalInput").ap()
    y_core = nc.dram_tensor("y_core", [tpc, B, DIM], bf16, kind="ExternalOutput").ap()

    with tile.TileContext(nc) as tc, ExitStack() as ctx:
        sb_pool = ctx.enter_context(tc.tile_pool(name="sb", bufs=1))
        ps_pool = ctx.enter_context(tc.tile_pool(name="ps", bufs=8, space="PSUM"))
        # one out slot per token: y stores drain the rings only after all
        # input pieces, so slot reuse would stall casts (and then PSUM/PE)
        out_pool = ctx.enter_context(tc.tile_pool(name="o", bufs=tpc))

        # alternate W pieces across the two HWDGE rings (a single ring tops
        # out near ~290 GB/s); each piece's x rides the opposite ring.
        x_tiles, w_tiles = [], []
        for pi, (p0, n) in enumerate(pieces):
            wring = nc.sync if pi % 2 == 0 else nc.scalar
            xring = nc.scalar if pi % 2 == 0 else nc.sync
            wt = sb_pool.tile([128, n * DIM], bf16, tag=f"w{pi}")
            wring.dma_start(wt[:], w_core[:, p0 * DIM:(p0 + n) * DIM])
            xt = sb_pool.tile([128, n * B], bf16, tag=f"x{pi}")
            xring.dma_start(xt[:], x_core[:, p0 * B:(p0 + n) * B])
            x_tiles.append(xt)
            w_tiles.append(wt)

        for j, c in enumerate(plan):
            psum = ps_pool.tile([B, DIM], f32)
            for ck in range(c):
                cc = tok_chunk0[j] + ck
                pi = piece_of_chunk[cc]
                lc = cc - pieces[pi][0]
                nc.tensor.matmul(
                    psum[:],
                    lhsT=x_tiles[pi][:, lc * B:(lc + 1) * B],
                    rhs=w_tiles[pi][:, lc * DIM:(lc + 1) * DIM],
                    start=(ck == 0),
                    stop=(ck == c - 1),
                )
            o_tile = out_pool.tile([B, DIM], bf16)
            if j % 2 == 0:
                nc.vector.tensor_copy(o_tile[:], psum[:])
            else:
                nc.scalar.copy(o_tile[:], psum[:])
            oring = nc.scalar if j % 2 == 0 else nc.sync
            oring.dma_start(y_core[j], o_tile[:])

    nc.compile()
    return nc


def _run_per_core(ncs, in_maps):
    """Per-device execution of 8 distinct single-core programs (adapted from
    bass2jax.run_bass_via_pjrt's single-core path)."""
    from concurrent.futures import ThreadPoolExecutor

    from concourse import mybir as mb
    from concourse.bass2jax import _bass_exec_p, install_neuronx_cc_hook

    install_neuronx_cc_hook()
    devices = jax.devices()[:8]

    def launch(k):
        nc = ncs[k]
        in_names, out_names, out_avals, zero_outs = [], [], [], []
        for alloc in nc.m.functions[0].allocations:
            if not isinstance(alloc, mb.MemoryLocationSet):
                continue
            name = alloc.memorylocations[0].name
            if alloc.kind == "ExternalInput":
                in_names.append(name)
            elif alloc.kind == "ExternalOutput":
                shape = tuple(alloc.tensor_shape)
                dtype = mb.dt.np(alloc.dtype)
                out_names.append(name)
                out_avals.append(jax.core.ShapedArray(shape, dtype))
                zero_outs.append(np.zeros(shape, dtype))
        n_params = len(in_names)
        all_names = tuple(in_names + out_names)
        donate = tuple(range(n_params, n_params + len(out_names)))

        def _body(*args):
            outs = _bass_exec_p.bind(
                *args,
                out_avals=tuple(out_avals),
                in_names=all_names,
                out_names=tuple(out_names),
                lowering_input_output_aliases=(),
                sim_require_finite=True,
                sim_require_nnan=True,
                nc=nc,
            )
            return tuple(outs)

        dev = devices[k]
        extras = {}
        for alloc in nc.m.functions[0].allocations:
            if (isinstance(alloc, mb.MemoryLocationSet)
                    and alloc.kind == "ExternalInput"):
                name = alloc.memorylocations[0].name
                if name not in in_maps[k]:
                    extras[name] = np.full(
                        tuple(alloc.tensor_shape), k, mb.dt.np(alloc.dtype))
        args = [jax.device_put(np.asarray(in_maps[k].get(n, extras.get(n))), dev)
                for n in in_names]
        args += [jax.device_put(z, dev) for z in zero_outs]
        out_arrs = jax.jit(_body, donate_argnums=donate, keep_unused=True)(*args)
        return out_names, out_arrs

    with ThreadPoolExecutor(max_workers=8) as ex:
        futs = [ex.submit(launch, k) for k in range(8)]
        handles = [f.result() for f in futs]
    return [
        {name: np.asarray(arr) for name, arr in zip(names, arrs)}
        for names, arrs in handles
    ]


LAST_RESULTS = None


def kernel(x, W, bias, idx_a, idx_b, idx_c, lens):
    global LAST_RESULTS
    x = np.asarray(x, np.float32)
    W = np.asarray(W, np.float32)
    bias = np.asarray(bias, np.float32)
    idx_a = np.asarray(idx_a, np.int32)
    idx_b = np.asarray(idx_b, np.int32)
    idx_c = np.asarray(idx_c, np.int32)
    lens = np.asarray(lens, np.int32)
    assert x.shape == (B, CIN, IMG, IMG) and W.shape == (T, Lmax, DIM)

    in_maps, plans, tok_lists = _shard(x, W, bias, idx_a, idx_b, idx_c, lens)
    if "ncs" not in _cache:
        _cache["ncs"] = [_build_program(plans[k]) for k in range(NCORES)]
    ncs = _cache["ncs"]

    hook = None
    trace = os.environ.get("BASS_TRACE") and not os.environ.get("BASS_NEVER_TRACE")
    if trace:
        from antenv.axon_hooks import get_axon_ntff_profile_hook

        hook = get_axon_ntff_profile_hook()
    if hook is not None:
        tmpdir = os.environ.get("KERNEL_TRACE_TMPDIR") or "/tmp/kernel_trace"
        os.makedirs(tmpdir, exist_ok=True)
        with hook(tmpdir, [0]):
            results = _run_per_core(ncs, in_maps)
        LAST_RESULTS = ("ntff", tmpdir, ncs[0])
    else:
        results = _run_per_core(ncs, in_maps)
        LAST_RESULTS = None

    y = np.empty((B, T, DIM), np.float32)
    for k in range(NCORES):
        y[:, tok_lists[k], :] = results[k]["y_core"].transpose(1, 0, 2).astype(
            np.float32)
    return y


# revision 21
# speedup vs baseline: 1.0927x; 1.0379x over previous
"""Trainium2 Bass kernel for DirectionalFreqEmbed (per-token gather + grouped GEMM).

Token-parallel across 8 NeuronCores, one compiled program per core, tokens
greedy-balanced by chunk count. The host shards the inputs into per-core
operand panels: for each token the gathered x values are packed densely into
ceil((len+1)/128) chunks of 128 l-slots ([128, 64] bf16 panels, batch on the
free axis, plus a ones-slot that folds the bias into the GEMM), and the
per-token W rows are permuted to match ([128, 384] bf16 per chunk, zero rows
on padding). The device program is then a pure streaming block-GEMM: per
token one W-tile DMA and C_t accumulated bf16 matmuls into PSUM, a bf16
cast-copy, and a store. W is read exactly once at its true ragged size
(sum(lens) rows, ~97 MB chip-wide instead of the 283 MB dense padding).

kernel(**inputs) takes FULL unsharded inputs and returns the FULL output.
"""
import os
import sys

import ml_dtypes
import numpy as np

for _p in ("/opt/trn_rl_repo", "/root/.axon_site/_ro/trn_rl_repo"):
    if os.path.isdir(_p) and _p not in sys.path:
        sys.path.insert(0, _p)

try:  # the staged antenv lacks axon_hooks; inject a functional stand-in
    import antenv.axon_hooks  # noqa: F401
except ImportError:
    import types as _types

    _hooks = _types.ModuleType("antenv.axon_hooks")
    _hooks._hook = None
    _hooks.get_axon_ntff_profile_hook = lambda: _hooks._hook
    _hooks.set_axon_ntff_profile_hook = lambda h: setattr(_hooks, "_hook", h)
    sys.modules["antenv.axon_hooks"] = _hooks

import jax
import concourse.bass as bass  # noqa: F401
import concourse.tile as tile
from concourse import bacc, mybir

IMG, CIN, DIM, B = 64, 30, 384, 64
T, Lmax = 240, 1452
NCORES = 8

bf16 = mybir.dt.bfloat16
f32 = mybir.dt.float32

_cache = {}


def _assign_tokens(lens):
    """Greedy LPT balance of tokens across cores by chunk count."""
    C = np.ceil((lens.astype(np.int64) + 1) / 128).astype(np.int64)
    order = np.argsort(-C, kind="stable")
    loads = [0] * NCORES
    toks = [[] for _ in range(NCORES)]
    for t in order:
        k = min(range(NCORES), key=lambda k: (loads[k], len(toks[k])))
        toks[k].append(int(t))
        loads[k] += int(C[t])
    # per-core order: 3 smallest first (fast pipeline start), then the rest
    # descending so the final token is small (short post-DMA tail chain).
    out = []
    for tk in toks:
        s = sorted(tk, key=lambda t: int(C[t]))
        out.append(s[:3] + s[3:][::-1])
    return out, C


def _shard(x, W, bias, idx_a, idx_b, idx_c, lens):
    """Per-core slot stream: each token padded to whole 128-slot chunks
    (PE base-partition rule requires chunk starts at partition 0), packed
    into [128, *] panels for full SBUF residency."""
    tok_lists, C = _assign_tokens(lens)
    in_maps, plans = [], []
    xbf = x.astype(np.float32)
    for k in range(NCORES):
        toks = tok_lists[k]
        nchunks = int(sum(C[t] for t in toks))
        xg = np.zeros((nchunks * 128, B), np.float32)
        wg = np.zeros((nchunks * 128, DIM), np.float32)
        plan = []
        s0 = 0
        for t in toks:
            L = int(lens[t])
            c = int(C[t])
            g = xbf[:, idx_c[t, :L], idx_a[t, :L], idx_b[t, :L]]  # [B, L]
            xg[s0:s0 + L] = g.T
            xg[s0 + L] = 1.0  # ones-slot -> bias row
            wg[s0:s0 + L] = W[t, :L]
            wg[s0 + L] = bias[t]
            plan.append(c)
            s0 += c * 128
        x_core = np.ascontiguousarray(
            xg.reshape(nchunks, 128, B).transpose(1, 0, 2).reshape(128, -1)
        ).astype(ml_dtypes.bfloat16)
        w_core = np.ascontiguousarray(
            wg.reshape(nchunks, 128, DIM).transpose(1, 0, 2).reshape(128, -1)
        ).astype(ml_dtypes.bfloat16)
        in_maps.append({"x_core": x_core, "w_core": w_core})
        plans.append(plan)
    return in_maps, plans, tok_lists


def _build_program(plan):
    from contextlib import ExitStack

    tpc = len(plan)
    nchunks = sum(plan)

    # W/x panels are fully SBUF-resident, loaded in a few large growing
    # pieces (piece boundaries on token boundaries) so PE can start early
    # while DMA streams at near line rate.
    tok_chunk0 = []
    c0 = 0
    for c in plan:
        tok_chunk0.append(c0)
        c0 += c
    # uniform ~8-chunk pieces, snapped to token boundaries: with strict ring
    # alternation the two rings deliver pieces nearly in lockstep, so the
    # in-order token consumption never waits long for a late piece.
    pieces, cur0, acc = [], 0, 0
    for j, c in enumerate(plan):
        acc += c
        want = 3 if len(pieces) < 2 else 8
        if acc >= want or j == tpc - 1:
            pieces.append((cur0, acc))
            cur0 += acc
            acc = 0
    piece_of_chunk = {}
    for pi, (p0, n) in enumerate(pieces):
        for cc in range(p0, p0 + n):
            piece_of_chunk[cc] = pi

    nc = bacc.Bacc("TRN2", target_bir_lowering=False, debug=False, num_devices=1)
    x_core = nc.dram_tensor("x_core", [128, nchunks * B], bf16,
                            kind="ExternalInput").ap()
    w_core = nc.dram_tensor("w_core", [128, nchunks * DIM], bf16,
                            kind="ExternalInput").ap()
    y_core = nc.dram_tensor("y_core", [tpc, B, DIM], bf16, kind="ExternalOutput").ap()

    with tile.TileContext(nc) as tc, ExitStack() as ctx:
        sb_pool = ctx.enter_context(tc.tile_pool(name="sb", bufs=1))
        ps_pool = ctx.enter_context(tc.tile_pool(name="ps", bufs=8, space="PSUM"))
        # one out slot per token: y stores drain the rings only after all
        # input pieces, so slot reuse would stall casts (and then PSUM/PE)
        out_pool = ctx.enter_context(tc.tile_pool(name="o", bufs=tpc))

        # alternate W pieces across the two HWDGE rings (a single ring tops
        # out near ~290 GB/s); each piece's x rides the opposite ring.
        x_tiles, w_tiles = [], []
        for pi, (p0, n) in enumerate(pieces):
            wring = nc.sync if pi % 2 == 0 else nc.scalar
            xring = nc.scalar if pi % 2 == 0 else nc.sync
            wt = sb_pool.tile([128, n * DIM], bf16, tag=f"w{pi}")
            wring.dma_start(wt[:], w_core[:, p0 * DIM:(p0 + n) * DIM])
            xt = sb_pool.tile([128, n * B], bf16, tag=f"x{pi}")
            xring.dma_start(xt[:], x_core[:, p0 * B:(p0 + n) * B])
            x_tiles.append(xt)
            w_tiles.append(wt)

        o_tiles = []
        for j, c in enumerate(plan):
            psum = ps_pool.tile([B, DIM], f32)
            for ck in range(c):
                cc = tok_chunk0[j] + ck
                pi = piece_of_chunk[cc]
                lc = cc - pieces[pi][0]
                nc.tensor.matmul(
                    psum[:],
                    lhsT=x_tiles[pi][:, lc * B:(lc + 1) * B],
                    rhs=w_tiles[pi][:, lc * DIM:(lc + 1) * DIM],
                    start=(ck == 0),
                    stop=(ck == c - 1),
                )
            o_tile = out_pool.tile([B, DIM], bf16)
            if j % 2 == 0:
                nc.vector.tensor_copy(o_tile[:], psum[:])
            else:
                nc.scalar.copy(o_tile[:], psum[:])
            o_tiles.append(o_tile)

        # y stores issue after every input piece: a store blocked on its cast
        # must never sit ahead of a piece DMA in an engine stream / DMA lane.
        for j, o_tile in enumerate(o_tiles):
            oring = nc.scalar if j % 2 == 0 else nc.sync
            oring.dma_start(y_core[j], o_tile[:])

    nc.compile()
    return nc


def _run_per_core(ncs, in_maps):
    """Per-device execution of 8 distinct single-core programs (adapted from
    bass2jax.run_bass_via_pjrt's single-core path)."""
    from concurrent.futures import ThreadPoolExecutor

    from concourse import mybir as mb
    from concourse.bass2jax import _bass_exec_p, install_neuronx_cc_hook

    install_neuronx_cc_hook()
    devices = jax.devices()[:8]

    def launch(k):
        nc = ncs[k]
        in_names, out_names, out_avals, zero_outs = [], [], [], []
        for alloc in nc.m.functions[0].allocations:
            if not isinstance(alloc, mb.MemoryLocationSet):
                continue
            name = alloc.memorylocations[0].name
            if alloc.kind == "ExternalInput":
                in_names.append(name)
            elif alloc.kind == "ExternalOutput":
                shape = tuple(alloc.tensor_shape)
                dtype = mb.dt.np(alloc.dtype)
                out_names.append(name)
                out_avals.append(jax.core.ShapedArray(shape, dtype))
                zero_outs.append(np.zeros(shape, dtype))
        n_params = len(in_names)
        all_names = tuple(in_names + out_names)
        donate = tuple(range(n_params, n_params + len(out_names)))

        def _body(*args):
            outs = _bass_exec_p.bind(
                *args,
                out_avals=tuple(out_avals),
                in_names=all_names,
                out_names=tuple(out_names),
                lowering_input_output_aliases=(),
                sim_require_finite=True,
                sim_require_nnan=True,
                nc=nc,
            )
            return tuple(outs)

        dev = devices[k]
        extras = {}
        for alloc in nc.m.functions[0].allocations:
            if (isinstance(alloc, mb.MemoryLocationSet)
                    and alloc.kind == "ExternalInput"):
                name = alloc.memorylocations[0].name
                if name not in in_maps[k]:
                    extras[name] = np.full(
                        tuple(alloc.tensor_shape), k, mb.dt.np(alloc.dtype))
        args = [jax.device_put(np.asarray(in_maps[k].get(n, extras.get(n))), dev)
                for n in in_names]
        args += [jax.device_put(z, dev) for z in zero_outs]
        out_arrs = jax.jit(_body, donate_argnums=donate, keep_unused=True)(*args)
        return out_names, out_arrs

    with ThreadPoolExecutor(max_workers=8) as ex:
        futs = [ex.submit(launch, k) for k in range(8)]
        handles = [f.result() for f in futs]
    return [
        {name: np.asarray(arr) for name, arr in zip(names, arrs)}
        for names, arrs in handles
    ]


LAST_RESULTS = None


def kernel(x, W, bias, idx_a, idx_b, idx_c, lens):
    global LAST_RESULTS
    x = np.asarray(x, np.float32)
    W = np.asarray(W, np.float32)
    bias = np.asarray(bias, np.float32)
    idx_a = np.asarray(idx_a, np.int32)
    idx_b = np.asarray(idx_b, np.int32)
    idx_c = np.asarray(idx_c, np.int32)
    lens = np.asarray(lens, np.int32)
    assert x.shape == (B, CIN, IMG, IMG) and W.shape == (T, Lmax, DIM)

    in_maps, plans, tok_lists = _shard(x, W, bias, idx_a, idx_b, idx_c, lens)
    if "ncs" not in _cache:
        _cache["ncs"] = [_build_program(plans[k]) for k in range(NCORES)]
    ncs = _cache["ncs"]

    hook = None
    trace = os.environ.get("BASS_TRACE") and not os.environ.get("BASS_NEVER_TRACE")
    if trace:
        from antenv.axon_hooks import get_axon_ntff_profile_hook

        hook = get_axon_ntff_profile_hook()
    if hook is not None:
        tmpdir = os.environ.get("KERNEL_TRACE_TMPDIR") or "/tmp/kernel_trace"
        os.makedirs(tmpdir, exist_ok=True)
        with hook(tmpdir, [0]):
            results = _run_per_core(ncs, in_maps)
        LAST_RESULTS = ("ntff", tmpdir, ncs[0])
    else:
        results = _run_per_core(ncs, in_maps)
        LAST_RESULTS = None

    y = np.empty((B, T, DIM), np.float32)
    for k in range(NCORES):
        y[:, tok_lists[k], :] = results[k]["y_core"].transpose(1, 0, 2).astype(
            np.float32)
    return y
